# revision 5
# baseline (speedup 1.0000x reference)
"""Grouped-experts SwiGLU MoE kernel for Trainium2 (8 NeuronCores).

Problem: x [8192, 2048] f32, 8 experts with w1/w3 [8, 1408, 2048] and
w2 [8, 2048, 1408]; tokens are expert-contiguous with a per-expert count
vector. out[t] = (silu(x_t @ w1_e.T) * (x_t @ w3_e.T)) @ w2_e.T for the
expert e owning token t.

Sharding: pure expert parallelism. Core e receives expert e's 1024-token
tile (dynamic-slice semantics of the reference) plus expert e's weights,
and computes the full SwiGLU MLP for that tile. No collectives.

All matmul operands are bf16 (host-cast): the PE streams bf16 at
1 elem/cell/cycle (same rate as fp32r) but DMA traffic and SBUF
footprint halve vs fp32. Accumulation stays fp32 in PSUM; rel err ~4e-3
(tolerance 2e-2).

Engine/DMA-ring assignment keeps every engine's queue clear of work that
would head-of-line block its consumers:
  - SP ring: stage-1 weight stream (w1+w3 packed, one DMA per h-tile)
    and stage-2 output stores. ACT ring: no DMAs at all — ACT only runs
    the sigmoid (+ half the stage-2 PSUM evacuations).
  - gpsimd (SWDGE) ring: the bulk x load (ladder-sized chunks so the
    first ps1/ps3 chains start ~3 us in) and the single 5.8 MB w2 load,
    both fully parallel to the SP weight stream.

Stage 1 computes hT [H, T] = silu(w1 xT) * (w3 xT) with the ps1/ps3
k-chains interleaved per contraction tile (PSUM [128h, 512t]); stage 2
is token-tile outer / dim-block inner with all of w2 resident
(PSUM [128t, 512d]), one output DMA per (tt, db) to keep the tail short.
"""

from contextlib import ExitStack

import ml_dtypes
import numpy as np

import concourse.bass as bass
import concourse.mybir as mybir
import concourse.tile as tile
from concourse import bacc
from concourse.bass import ts
from concourse.bass_utils import run_bass_kernel_spmd

F32 = mybir.dt.float32
BF16 = mybir.dt.bfloat16
NP_BF16 = ml_dtypes.bfloat16

N_TOKENS = 8192
DIM = 2048
HIDDEN = 1408
N_EXPERTS = 8
CAP = N_TOKENS // N_EXPERTS  # 1024 tokens per core
P = 128
KD = DIM // P  # 16 contraction tiles, stage 1
KH = HIDDEN // P  # 11 contraction tiles, stage 2
TB = 512  # token-block (stage-1 moving free dim)
DB = 512  # dim-block (stage-2 moving free dim)
N_TB = CAP // TB  # 2
N_DB = DIM // DB  # 4
N_TT = CAP // P  # 8 token tiles (stage-2 stationary)
X_CHUNKS = ((0, 1), (1, 4), (4, 10), (10, KD))  # ko ladder for the x load

_CACHED_NC = None


def _build_nc(reps=1):
    nc = bacc.Bacc("TRN2", debug=False)
    xQ = nc.dram_tensor("xQ", [P, KD, CAP], BF16, kind="ExternalInput").ap()
    # w1 and w3 packed: one DMA per h-tile feeds both k-chains
    wQ = nc.dram_tensor("wQ", [KH, P, 2, KD, P], BF16, kind="ExternalInput").ap()
    w2Q = nc.dram_tensor("w2Q", [P, KH, DIM], BF16, kind="ExternalInput").ap()
    out = nc.dram_tensor("out", [CAP, DIM], F32, kind="ExternalOutput").ap()

    with tile.TileContext(nc) as tc, ExitStack() as ctx:
        xpool = ctx.enter_context(tc.tile_pool(name="xpool", bufs=1))
        hpool = ctx.enter_context(tc.tile_pool(name="hpool", bufs=1))
        wpool = ctx.enter_context(tc.tile_pool(name="wpool", bufs=3))
        w2pool = ctx.enter_context(tc.tile_pool(name="w2pool", bufs=1))
        tmppool = ctx.enter_context(tc.tile_pool(name="tmppool", bufs=3))
        opool = ctx.enter_context(tc.tile_pool(name="opool", bufs=3))
        pspool = ctx.enter_context(tc.tile_pool(name="pspool", bufs=2, space="PSUM"))

        for _rep in range(reps):
            x_sb = xpool.tile([P, KD, CAP], BF16)
            h_sb = hpool.tile([P, KH, CAP], BF16)
            w2_sb = w2pool.tile([P, KH, DIM], BF16)

            # Prologue: ht=0 weights on SP (w1 half first so the ps1
            # chain can start as early as possible); x ladder + w2 on the
            # gpsimd ring, fully parallel to the SP weight stream.
            w_first = wpool.tile([P, 2, KD, P], BF16, tag="w")
            nc.sync.dma_start(w_first[:, 0], wQ[0, :, 0])
            nc.sync.dma_start(w_first[:, 1], wQ[0, :, 1])
            for lo, hi in X_CHUNKS:
                nc.gpsimd.dma_start(x_sb[:, lo:hi], xQ[:, lo:hi])
            nc.gpsimd.dma_start(w2_sb[:], w2Q[:])

            # Stage 1: hT = silu(x @ w1.T).T * (x @ w3.T).T
            for ht in range(KH):
                if ht == 0:
                    w_sb = w_first
                else:
                    w_sb = wpool.tile([P, 2, KD, P], BF16, tag="w")
                    nc.sync.dma_start(w_sb[:], wQ[ht])
                for tb in range(N_TB):
                    ps1 = pspool.tile([P, TB], F32, tag="ps1")
                    ps3 = pspool.tile([P, TB], F32, tag="ps3")
                    for k in range(KD):
                        nc.tensor.matmul(
                            ps1[:], w_sb[:, 0, k], x_sb[:, k, ts(tb, TB)],
                            start=(k == 0), stop=(k == KD - 1),
                        )
                        nc.tensor.matmul(
                            ps3[:], w_sb[:, 1, k], x_sb[:, k, ts(tb, TB)],
                            start=(k == 0), stop=(k == KD - 1),
                        )
                    sil = tmppool.tile([P, TB], F32, tag="sil")
                    nc.scalar.activation(
                        sil[:], ps1[:], mybir.ActivationFunctionType.Silu
                    )
                    nc.vector.tensor_mul(h_sb[:, ht, ts(tb, TB)], sil[:], ps3[:])

            # Stage 2: out = hT.T @ w2.T — stationary hT token-tiles,
            # moving w2 dim-blocks, all of w2 resident in SBUF.
            for tt in range(N_TT):
                for db in range(N_DB):
                    ps = pspool.tile([P, DB], F32, tag="ps2")
                    for k in range(KH):
                        nc.tensor.matmul(
                            ps[:], h_sb[:, k, ts(tt, P)], w2_sb[:, k, ts(db, DB)],
                            start=(k == 0), stop=(k == KH - 1),
                        )
                    ot = opool.tile([P, DB], F32, tag="ot")
                    if db % 2 == 0:
                        nc.vector.tensor_copy(ot[:], ps[:])
                    else:
                        nc.scalar.activation(
                            ot[:], ps[:], mybir.ActivationFunctionType.Copy
                        )
                    nc.sync.dma_start(out[ts(tt, P), ts(db, DB)], ot[:])

    nc.compile()
    return nc


def _get_nc():
    global _CACHED_NC
    if _CACHED_NC is None:
        _CACHED_NC = _build_nc()
    return _CACHED_NC


def _pack_inputs(x, w1, w2, w3, read_starts):
    """Per-core input dicts, bf16, DMA-optimal (partition-major) layouts."""
    in_maps = []
    for e in range(N_EXPERTS):
        s = int(read_starts[e])
        xe = x[s : s + CAP].astype(NP_BF16)  # [CAP, DIM]
        xQ = np.ascontiguousarray(xe.T.reshape(KD, P, CAP).transpose(1, 0, 2))
        w1Q = w1[e].astype(NP_BF16).T.reshape(KD, P, KH, P).transpose(2, 1, 0, 3)
        w3Q = w3[e].astype(NP_BF16).T.reshape(KD, P, KH, P).transpose(2, 1, 0, 3)
        wQ = np.ascontiguousarray(np.stack([w1Q, w3Q], axis=2))  # [KH,P,2,KD,P]
        w2Q = np.ascontiguousarray(
            w2[e].astype(NP_BF16).T.reshape(KH, P, DIM).transpose(1, 0, 2)
        )
        in_maps.append({"xQ": xQ, "wQ": wQ, "w2Q": w2Q})
    return in_maps


def kernel(x, num_tokens_per_expert, w1, w2, w3):
    x = np.ascontiguousarray(np.asarray(x, dtype=np.float32))
    w1 = np.asarray(w1, dtype=np.float32)
    w2 = np.asarray(w2, dtype=np.float32)
    w3 = np.asarray(w3, dtype=np.float32)
    counts = np.asarray(num_tokens_per_expert).astype(np.int64)

    offsets = np.cumsum(counts)
    starts = offsets - counts
    # jax.lax.dynamic_slice clamps the read start so the slice is in-bounds.
    read_starts = np.clip(starts, 0, N_TOKENS - CAP)

    in_maps = _pack_inputs(x, w1, w2, w3, read_starts)
    nc = _get_nc()
    res = run_bass_kernel_spmd(nc, in_maps, core_ids=list(range(N_EXPERTS)))
    ye = [res.results[e]["out"] for e in range(N_EXPERTS)]

    if np.all(counts == CAP):
        # balanced routing: per-expert tiles are disjoint and exactly cover x
        return np.concatenate(ye, axis=0)

    # general case: mask invalid slots, scatter-add to clipped positions
    y = np.zeros((N_TOKENS, DIM), np.float32)
    slot = np.arange(CAP)
    for e in range(N_EXPERTS):
        valid = slot < counts[e]
        pos = np.clip(starts[e] + slot, 0, N_TOKENS - 1)
        np.add.at(y, pos, np.where(valid[:, None], ye[e], 0.0))
    return y


# revision 7
# speedup vs baseline: 16.5791x; 16.5791x over previous
"""Grouped-experts SwiGLU MoE kernel for Trainium2 (8 NeuronCores).

Problem: x [8192, 2048] f32, 8 experts with w1/w3 [8, 1408, 2048] and
w2 [8, 2048, 1408]; tokens are expert-contiguous with a per-expert count
vector. out[t] = (silu(x_t @ w1_e.T) * (x_t @ w3_e.T)) @ w2_e.T for the
expert e owning token t.

Sharding: pure expert parallelism. Core e receives expert e's 1024-token
tile (dynamic-slice semantics of the reference) plus expert e's weights,
and computes the full SwiGLU MLP for that tile. No collectives.

All matmul operands are bf16 (host-cast): the PE streams bf16 at
1 elem/cell/cycle (same rate as fp32r) but DMA traffic and SBUF
footprint halve vs fp32. Accumulation stays fp32 in PSUM; rel err ~4e-3
(tolerance 2e-2).

Engine/DMA-ring assignment keeps every engine's queue clear of work that
would head-of-line block its consumers:
  - SP ring: stage-1 weight stream (w1+w3 packed, one DMA per h-tile)
    and stage-2 output stores. ACT ring: no DMAs at all — ACT only runs
    the sigmoid (+ half the stage-2 PSUM evacuations).
  - gpsimd (SWDGE) ring: the bulk x load (ladder-sized chunks so the
    first ps1/ps3 chains start ~3 us in) and the single 5.8 MB w2 load,
    both fully parallel to the SP weight stream.

Stage 1 computes hT [H, T] = silu(w1 xT) * (w3 xT) with the ps1/ps3
k-chains interleaved per contraction tile (PSUM [128h, 512t]); stage 2
is token-tile outer / dim-block inner with all of w2 resident
(PSUM [128t, 512d]), one output DMA per (tt, db) to keep the tail short.
"""

from contextlib import ExitStack

import ml_dtypes
import numpy as np

import concourse.bass as bass
import concourse.mybir as mybir
import concourse.tile as tile
from concourse import bacc
from concourse.bass import ts
from concourse.bass_utils import run_bass_kernel_spmd

F32 = mybir.dt.float32
BF16 = mybir.dt.bfloat16
NP_BF16 = ml_dtypes.bfloat16

N_TOKENS = 8192
DIM = 2048
HIDDEN = 1408
N_EXPERTS = 8
CAP = N_TOKENS // N_EXPERTS  # 1024 tokens per core
P = 128
KD = DIM // P  # 16 contraction tiles, stage 1
KH = HIDDEN // P  # 11 contraction tiles, stage 2
TB = 512  # token-block (stage-1 moving free dim)
DB = 512  # dim-block (stage-2 moving free dim)
N_TB = CAP // TB  # 2
N_DB = DIM // DB  # 4
N_TT = CAP // P  # 8 token tiles (stage-2 stationary)
X_CHUNKS = ((0, 1), (1, 4), (4, 10), (10, KD))  # ko ladder for the x load

_CACHED_NC = None


def _build_nc(reps=1):
    nc = bacc.Bacc("TRN2", debug=False)
    xQ = nc.dram_tensor("xQ", [P, KD, CAP], BF16, kind="ExternalInput").ap()
    # w1 and w3 packed: one DMA per h-tile feeds both k-chains
    wQ = nc.dram_tensor("wQ", [KH, P, 2, KD, P], BF16, kind="ExternalInput").ap()
    w2Q = nc.dram_tensor("w2Q", [P, KH, DIM], BF16, kind="ExternalInput").ap()
    out = nc.dram_tensor("out", [CAP, DIM], F32, kind="ExternalOutput").ap()

    with tile.TileContext(nc) as tc, ExitStack() as ctx:
        xpool = ctx.enter_context(tc.tile_pool(name="xpool", bufs=1))
        hpool = ctx.enter_context(tc.tile_pool(name="hpool", bufs=1))
        wpool = ctx.enter_context(tc.tile_pool(name="wpool", bufs=3))
        w2pool = ctx.enter_context(tc.tile_pool(name="w2pool", bufs=1))
        tmppool = ctx.enter_context(tc.tile_pool(name="tmppool", bufs=3))
        opool = ctx.enter_context(tc.tile_pool(name="opool", bufs=3))
        pspool = ctx.enter_context(tc.tile_pool(name="pspool", bufs=2, space="PSUM"))

        for _rep in range(reps):
            x_sb = xpool.tile([P, KD, CAP], BF16)
            h_sb = hpool.tile([P, KH, CAP], BF16)
            w2_sb = w2pool.tile([P, KH, DIM], BF16)

            # Prologue: ht=0 weights on SP as two separate tiles so the
            # first ps1 chain waits only on the w1 half; x ladder + w2 on
            # the gpsimd ring, fully parallel to the SP weight stream.
            w1_first = wpool.tile([P, KD, P], BF16, tag="w1a", bufs=1)
            nc.sync.dma_start(w1_first[:], wQ[0, :, 0])
            w3_first = wpool.tile([P, KD, P], BF16, tag="w3a", bufs=1)
            nc.sync.dma_start(w3_first[:], wQ[0, :, 1])
            for lo, hi in X_CHUNKS:
                nc.gpsimd.dma_start(x_sb[:, lo:hi], xQ[:, lo:hi])
            nc.gpsimd.dma_start(w2_sb[:], w2Q[:])
            if _rep == 0:
                # PE warmup on a zeroed tile while the first DMAs land:
                # ramps the clock (HAM) so the real chains run full-speed.
                # Junk results land in the ps1 bank the real k=0 matmul
                # clears with start=True.
                warm = tmppool.tile([P, TB], BF16, tag="warm", bufs=1)
                nc.vector.memset(warm[:], 0.0)

            # Stage 1: hT = silu(x @ w1.T).T * (x @ w3.T).T
            for ht in range(KH):
                if ht == 0:
                    w1s, w3s = w1_first, w3_first
                else:
                    w_sb = wpool.tile([P, 2, KD, P], BF16, tag="w")
                    nc.sync.dma_start(w_sb[:], wQ[ht])
                    w1s, w3s = w_sb[:, 0], w_sb[:, 1]
                for tb in range(N_TB):
                    ps1 = pspool.tile([P, TB], F32, tag="ps1")
                    ps3 = pspool.tile([P, TB], F32, tag="ps3")
                    if _rep == 0 and ht == 0 and tb == 0:
                        for _ in range(3):
                            nc.tensor.matmul(
                                ps1[:], warm[:, 0:P], warm[:],
                                start=True, stop=True,
                            )
                    for k in range(KD):
                        nc.tensor.matmul(
                            ps1[:], w1s[:, k], x_sb[:, k, ts(tb, TB)],
                            start=(k == 0), stop=(k == KD - 1),
                        )
                        nc.tensor.matmul(
                            ps3[:], w3s[:, k], x_sb[:, k, ts(tb, TB)],
                            start=(k == 0), stop=(k == KD - 1),
                        )
                    sil = tmppool.tile([P, TB], F32, tag="sil")
                    nc.scalar.activation(
                        sil[:], ps1[:], mybir.ActivationFunctionType.Silu
                    )
                    nc.vector.tensor_mul(h_sb[:, ht, ts(tb, TB)], sil[:], ps3[:])

            # Stage 2: out = hT.T @ w2.T — stationary hT token-tiles,
            # moving w2 dim-blocks, all of w2 resident in SBUF.
            for tt in range(N_TT):
                for db in range(N_DB):
                    ps = pspool.tile([P, DB], F32, tag="ps2")
                    for k in range(KH):
                        nc.tensor.matmul(
                            ps[:], h_sb[:, k, ts(tt, P)], w2_sb[:, k, ts(db, DB)],
                            start=(k == 0), stop=(k == KH - 1),
                        )
                    ot = opool.tile([P, DB], F32, tag="ot")
                    if db % 2 == 0:
                        nc.vector.tensor_copy(ot[:], ps[:])
                    else:
                        nc.scalar.activation(
                            ot[:], ps[:], mybir.ActivationFunctionType.Copy
                        )
                    nc.sync.dma_start(out[ts(tt, P), ts(db, DB)], ot[:])

    nc.compile()
    return nc


def _get_nc():
    global _CACHED_NC
    if _CACHED_NC is None:
        _CACHED_NC = _build_nc()
    return _CACHED_NC


def _pack_inputs(x, w1, w2, w3, read_starts):
    """Per-core input dicts, bf16, DMA-optimal (partition-major) layouts."""
    in_maps = []
    for e in range(N_EXPERTS):
        s = int(read_starts[e])
        xe = x[s : s + CAP].astype(NP_BF16)  # [CAP, DIM]
        xQ = np.ascontiguousarray(xe.T.reshape(KD, P, CAP).transpose(1, 0, 2))
        w1Q = w1[e].astype(NP_BF16).T.reshape(KD, P, KH, P).transpose(2, 1, 0, 3)
        w3Q = w3[e].astype(NP_BF16).T.reshape(KD, P, KH, P).transpose(2, 1, 0, 3)
        wQ = np.ascontiguousarray(np.stack([w1Q, w3Q], axis=2))  # [KH,P,2,KD,P]
        w2Q = np.ascontiguousarray(
            w2[e].astype(NP_BF16).T.reshape(KH, P, DIM).transpose(1, 0, 2)
        )
        in_maps.append({"xQ": xQ, "wQ": wQ, "w2Q": w2Q})
    return in_maps


def kernel(x, num_tokens_per_expert, w1, w2, w3):
    x = np.ascontiguousarray(np.asarray(x, dtype=np.float32))
    w1 = np.asarray(w1, dtype=np.float32)
    w2 = np.asarray(w2, dtype=np.float32)
    w3 = np.asarray(w3, dtype=np.float32)
    counts = np.asarray(num_tokens_per_expert).astype(np.int64)

    offsets = np.cumsum(counts)
    starts = offsets - counts
    # jax.lax.dynamic_slice clamps the read start so the slice is in-bounds.
    read_starts = np.clip(starts, 0, N_TOKENS - CAP)

    in_maps = _pack_inputs(x, w1, w2, w3, read_starts)
    nc = _get_nc()
    res = run_bass_kernel_spmd(nc, in_maps, core_ids=list(range(N_EXPERTS)))
    ye = [res.results[e]["out"] for e in range(N_EXPERTS)]

    if np.all(counts == CAP):
        # balanced routing: per-expert tiles are disjoint and exactly cover x
        return np.concatenate(ye, axis=0)

    # general case: mask invalid slots, scatter-add to clipped positions
    y = np.zeros((N_TOKENS, DIM), np.float32)
    slot = np.arange(CAP)
    for e in range(N_EXPERTS):
        valid = slot < counts[e]
        pos = np.clip(starts[e] + slot, 0, N_TOKENS - 1)
        np.add.at(y, pos, np.where(valid[:, None], ye[e], 0.0))
    return y


# revision 8
# speedup vs baseline: 21.6113x; 1.3035x over previous
"""Grouped-experts SwiGLU MoE kernel for Trainium2 (8 NeuronCores).

Problem: x [8192, 2048] f32, 8 experts with w1/w3 [8, 1408, 2048] and
w2 [8, 2048, 1408]; tokens are expert-contiguous with a per-expert count
vector. out[t] = (silu(x_t @ w1_e.T) * (x_t @ w3_e.T)) @ w2_e.T for the
expert e owning token t.

Sharding: pure expert parallelism. Core e receives expert e's 1024-token
tile (dynamic-slice semantics of the reference) plus expert e's weights,
and computes the full SwiGLU MLP for that tile. No collectives.

All matmul operands are bf16 (host-cast): the PE streams bf16 at
1 elem/cell/cycle (same rate as fp32r) but DMA traffic and SBUF
footprint halve vs fp32. Accumulation stays fp32 in PSUM; rel err ~4e-3
(tolerance 2e-2).

Engine/DMA-ring assignment keeps every engine's queue clear of work that
would head-of-line block its consumers:
  - SP ring: stage-1 weight stream (w1+w3 packed, one DMA per h-tile)
    and stage-2 output stores. ACT ring: no DMAs at all — ACT only runs
    the sigmoid (+ half the stage-2 PSUM evacuations).
  - gpsimd (SWDGE) ring: the bulk x load (ladder-sized chunks so the
    first ps1/ps3 chains start ~3 us in) and the single 5.8 MB w2 load,
    both fully parallel to the SP weight stream.

Stage 1 computes hT [H, T] = silu(w1 xT) * (w3 xT) with the ps1/ps3
k-chains interleaved per contraction tile (PSUM [128h, 512t]); stage 2
is token-tile outer / dim-block inner with all of w2 resident
(PSUM [128t, 512d]), one output DMA per (tt, db) to keep the tail short.
"""

from contextlib import ExitStack

import ml_dtypes
import numpy as np

import concourse.bass as bass
import concourse.mybir as mybir
import concourse.tile as tile
from concourse import bacc
from concourse.bass import ts
from concourse.bass_utils import run_bass_kernel_spmd

F32 = mybir.dt.float32
BF16 = mybir.dt.bfloat16
NP_BF16 = ml_dtypes.bfloat16

N_TOKENS = 8192
DIM = 2048
HIDDEN = 1408
N_EXPERTS = 8
CAP = N_TOKENS // N_EXPERTS  # 1024 tokens per core
P = 128
KD = DIM // P  # 16 contraction tiles, stage 1
KH = HIDDEN // P  # 11 contraction tiles, stage 2
TB = 512  # token-block (stage-1 moving free dim)
DB = 512  # dim-block (stage-2 moving free dim)
N_TB = CAP // TB  # 2
N_DB = DIM // DB  # 4
N_TT = CAP // P  # 8 token tiles (stage-2 stationary)
X_CHUNKS = ((0, 1), (1, 4), (4, 10), (10, KD))  # ko ladder for the x load

_CACHED_NC = None


def _build_nc(reps=1):
    nc = bacc.Bacc("TRN2", debug=False)
    xQ = nc.dram_tensor("xQ", [P, KD, CAP], BF16, kind="ExternalInput").ap()
    # w1 and w3 packed: one DMA per h-tile feeds both k-chains
    wQ = nc.dram_tensor("wQ", [KH, P, 2, KD, P], BF16, kind="ExternalInput").ap()
    w2Q = nc.dram_tensor("w2Q", [P, KH, DIM], BF16, kind="ExternalInput").ap()
    out = nc.dram_tensor("out", [CAP, DIM], BF16, kind="ExternalOutput").ap()

    with tile.TileContext(nc) as tc, ExitStack() as ctx:
        xpool = ctx.enter_context(tc.tile_pool(name="xpool", bufs=2))
        hpool = ctx.enter_context(tc.tile_pool(name="hpool", bufs=1))
        wpool = ctx.enter_context(tc.tile_pool(name="wpool", bufs=3))
        w2pool = ctx.enter_context(tc.tile_pool(name="w2pool", bufs=1))
        tmppool = ctx.enter_context(tc.tile_pool(name="tmppool", bufs=3))
        opool = ctx.enter_context(tc.tile_pool(name="opool", bufs=3))
        pspool = ctx.enter_context(tc.tile_pool(name="pspool", bufs=2, space="PSUM"))

        for _rep in range(reps):
            x_sb = xpool.tile([P, KD, CAP], BF16)
            h_sb = hpool.tile([P, KH, CAP], BF16)
            w2_sb = w2pool.tile([P, KH, DIM], BF16)

            # Prologue: ht=0 weights on SP as two separate tiles so the
            # first ps1 chain waits only on the w1 half; x ladder + w2 on
            # the gpsimd ring, fully parallel to the SP weight stream.
            w1_first = wpool.tile([P, KD, P], BF16, tag="w1a", bufs=1)
            nc.sync.dma_start(w1_first[:], wQ[0, :, 0])
            w3_first = wpool.tile([P, KD, P], BF16, tag="w3a", bufs=1)
            nc.sync.dma_start(w3_first[:], wQ[0, :, 1])
            for lo, hi in X_CHUNKS:
                nc.gpsimd.dma_start(x_sb[:, lo:hi], xQ[:, lo:hi])
            nc.gpsimd.dma_start(w2_sb[:], w2Q[:])
            if _rep == 0:
                # PE warmup on a zeroed tile while the first DMAs land:
                # ramps the clock (HAM) so the real chains run full-speed.
                # Junk results land in the ps1 bank the real k=0 matmul
                # clears with start=True.
                warm = tmppool.tile([P, TB], BF16, tag="warm", bufs=1)
                nc.vector.memset(warm[:], 0.0)

            # Stage 1: hT = silu(x @ w1.T).T * (x @ w3.T).T
            for ht in range(KH):
                if ht == 0:
                    w1s, w3s = w1_first, w3_first
                else:
                    w_sb = wpool.tile([P, 2, KD, P], BF16, tag="w")
                    nc.sync.dma_start(w_sb[:], wQ[ht])
                    w1s, w3s = w_sb[:, 0], w_sb[:, 1]
                for tb in range(N_TB):
                    ps1 = pspool.tile([P, TB], F32, tag="ps1", bufs=3)
                    ps3 = pspool.tile([P, TB], F32, tag="ps3", bufs=3)
                    if _rep == 0 and ht == 0 and tb == 0:
                        for _ in range(3):
                            nc.tensor.matmul(
                                ps1[:], warm[:, 0:P], warm[:],
                                start=True, stop=True,
                            )
                    for k in range(KD):
                        nc.tensor.matmul(
                            ps1[:], w1s[:, k], x_sb[:, k, ts(tb, TB)],
                            start=(k == 0), stop=(k == KD - 1),
                        )
                        nc.tensor.matmul(
                            ps3[:], w3s[:, k], x_sb[:, k, ts(tb, TB)],
                            start=(k == 0), stop=(k == KD - 1),
                        )
                    sil = tmppool.tile([P, TB], F32, tag="sil")
                    nc.scalar.activation(
                        sil[:], ps1[:], mybir.ActivationFunctionType.Silu
                    )
                    nc.vector.tensor_mul(h_sb[:, ht, ts(tb, TB)], sil[:], ps3[:])

            # Stage 2: out = hT.T @ w2.T — stationary hT token-tiles,
            # moving w2 dim-blocks, all of w2 resident in SBUF.
            for tt in range(N_TT):
                for db in range(N_DB):
                    ps = pspool.tile([P, DB], F32, tag="ps2")
                    for k in range(KH):
                        nc.tensor.matmul(
                            ps[:], h_sb[:, k, ts(tt, P)], w2_sb[:, k, ts(db, DB)],
                            start=(k == 0), stop=(k == KH - 1),
                        )
                    ot = opool.tile([P, DB], BF16, tag="ot")
                    if db % 2 == 0:
                        nc.vector.tensor_copy(ot[:], ps[:])
                    else:
                        nc.scalar.activation(
                            ot[:], ps[:], mybir.ActivationFunctionType.Copy
                        )
                    nc.gpsimd.dma_start(out[ts(tt, P), ts(db, DB)], ot[:])

    nc.compile()
    return nc


def _get_nc():
    global _CACHED_NC
    if _CACHED_NC is None:
        _CACHED_NC = _build_nc()
    return _CACHED_NC


def _pack_inputs(x, w1, w2, w3, read_starts):
    """Per-core input dicts, bf16, DMA-optimal (partition-major) layouts."""
    in_maps = []
    for e in range(N_EXPERTS):
        s = int(read_starts[e])
        xe = x[s : s + CAP].astype(NP_BF16)  # [CAP, DIM]
        xQ = np.ascontiguousarray(xe.T.reshape(KD, P, CAP).transpose(1, 0, 2))
        w1Q = w1[e].astype(NP_BF16).T.reshape(KD, P, KH, P).transpose(2, 1, 0, 3)
        w3Q = w3[e].astype(NP_BF16).T.reshape(KD, P, KH, P).transpose(2, 1, 0, 3)
        wQ = np.ascontiguousarray(np.stack([w1Q, w3Q], axis=2))  # [KH,P,2,KD,P]
        w2Q = np.ascontiguousarray(
            w2[e].astype(NP_BF16).T.reshape(KH, P, DIM).transpose(1, 0, 2)
        )
        in_maps.append({"xQ": xQ, "wQ": wQ, "w2Q": w2Q})
    return in_maps


def kernel(x, num_tokens_per_expert, w1, w2, w3):
    x = np.ascontiguousarray(np.asarray(x, dtype=np.float32))
    w1 = np.asarray(w1, dtype=np.float32)
    w2 = np.asarray(w2, dtype=np.float32)
    w3 = np.asarray(w3, dtype=np.float32)
    counts = np.asarray(num_tokens_per_expert).astype(np.int64)

    offsets = np.cumsum(counts)
    starts = offsets - counts
    # jax.lax.dynamic_slice clamps the read start so the slice is in-bounds.
    read_starts = np.clip(starts, 0, N_TOKENS - CAP)

    in_maps = _pack_inputs(x, w1, w2, w3, read_starts)
    nc = _get_nc()
    res = run_bass_kernel_spmd(nc, in_maps, core_ids=list(range(N_EXPERTS)))
    ye = [np.asarray(res.results[e]["out"]).astype(np.float32) for e in range(N_EXPERTS)]

    if np.all(counts == CAP):
        # balanced routing: per-expert tiles are disjoint and exactly cover x
        return np.concatenate(ye, axis=0)

    # general case: mask invalid slots, scatter-add to clipped positions
    y = np.zeros((N_TOKENS, DIM), np.float32)
    slot = np.arange(CAP)
    for e in range(N_EXPERTS):
        valid = slot < counts[e]
        pos = np.clip(starts[e] + slot, 0, N_TOKENS - 1)
        np.add.at(y, pos, np.where(valid[:, None], ye[e], 0.0))
    return y


# revision 11
# speedup vs baseline: 25.4429x; 1.1773x over previous
"""Grouped-experts SwiGLU MoE kernel for Trainium2 (8 NeuronCores).

Problem: x [8192, 2048] f32, 8 experts with w1/w3 [8, 1408, 2048] and
w2 [8, 2048, 1408]; tokens are expert-contiguous with a per-expert count
vector. out[t] = (silu(x_t @ w1_e.T) * (x_t @ w3_e.T)) @ w2_e.T for the
expert e owning token t.

Sharding: pure expert parallelism. Core e receives expert e's 1024-token
tile (dynamic-slice semantics of the reference) plus expert e's weights,
and computes the full SwiGLU MLP for that tile. No collectives.

All matmul operands are bf16 (host-cast): the PE streams bf16 at
1 elem/cell/cycle (same rate as fp32r) but DMA traffic and SBUF
footprint halve vs fp32. Accumulation stays fp32 in PSUM; rel err ~4e-3
(tolerance 2e-2).

Engine/DMA-ring assignment keeps every engine's queue clear of work that
would head-of-line block its consumers:
  - SP ring: stage-1 weight stream (w1+w3 packed, one DMA per h-tile)
    and stage-2 output stores. ACT ring: no DMAs at all — ACT only runs
    the sigmoid (+ half the stage-2 PSUM evacuations).
  - gpsimd (SWDGE) ring: the bulk x load (ladder-sized chunks so the
    first ps1/ps3 chains start ~3 us in) and the single 5.8 MB w2 load,
    both fully parallel to the SP weight stream.

Stage 1 computes hT [H, T] = silu(w1 xT) * (w3 xT) with the ps1/ps3
k-chains interleaved per contraction tile (PSUM [128h, 512t]); stage 2
is token-tile outer / dim-block inner with all of w2 resident
(PSUM [128t, 512d]), one output DMA per (tt, db) to keep the tail short.
"""

from contextlib import ExitStack

import ml_dtypes
import numpy as np

import concourse.bass as bass
import concourse.mybir as mybir
import concourse.tile as tile
from concourse import bacc
from concourse.bass import ts
from concourse.bass_utils import run_bass_kernel_spmd

F32 = mybir.dt.float32
BF16 = mybir.dt.bfloat16
NP_BF16 = ml_dtypes.bfloat16

N_TOKENS = 8192
DIM = 2048
HIDDEN = 1408
N_EXPERTS = 8
CAP = N_TOKENS // N_EXPERTS  # 1024 tokens per core
P = 128
KD = DIM // P  # 16 contraction tiles, stage 1
KH = HIDDEN // P  # 11 contraction tiles, stage 2
TB = 512  # token-block (stage-1 moving free dim)
DB = 512  # dim-block (stage-2 moving free dim)
N_TB = CAP // TB  # 2
N_DB = DIM // DB  # 4
N_TT = CAP // P  # 8 token tiles (stage-2 stationary)
X_CHUNKS = ((0, 1), (1, 4), (4, 10), (10, KD))  # ko ladder for the x load

_CACHED_NC = None


def _build_nc(reps=1):
    nc = bacc.Bacc("TRN2", debug=False)
    xQ = nc.dram_tensor("xQ", [P, KD, CAP], BF16, kind="ExternalInput").ap()
    # w1 and w3 packed: one DMA per h-tile feeds both k-chains
    wQ = nc.dram_tensor("wQ", [KH, P, 2, KD, P], BF16, kind="ExternalInput").ap()
    w2Q = nc.dram_tensor("w2Q", [P, KH, DIM], BF16, kind="ExternalInput").ap()
    out = nc.dram_tensor("out", [CAP, DIM], BF16, kind="ExternalOutput").ap()

    with tile.TileContext(nc) as tc, ExitStack() as ctx:
        xpool = ctx.enter_context(tc.tile_pool(name="xpool", bufs=2))
        hpool = ctx.enter_context(tc.tile_pool(name="hpool", bufs=1))
        wpool = ctx.enter_context(tc.tile_pool(name="wpool", bufs=3))
        w2pool = ctx.enter_context(tc.tile_pool(name="w2pool", bufs=1))
        tmppool = ctx.enter_context(tc.tile_pool(name="tmppool", bufs=3))
        opool = ctx.enter_context(tc.tile_pool(name="opool", bufs=3))
        pspool = ctx.enter_context(tc.tile_pool(name="pspool", bufs=2, space="PSUM"))

        for _rep in range(reps):
            x_sb = xpool.tile([P, KD, CAP], BF16)
            h_sb = hpool.tile([P, KH, CAP], BF16)
            w2_sb = w2pool.tile([P, KH, DIM], BF16)

            # Prologue: ht=0 weights on SP as two separate tiles so the
            # first ps1 chain waits only on the w1 half; x ladder + w2 on
            # the gpsimd ring, fully parallel to the SP weight stream.
            w1_first = wpool.tile([P, KD, P], BF16, tag="w1a", bufs=1)
            nc.sync.dma_start(w1_first[:], wQ[0, :, 0])
            w3_first = wpool.tile([P, KD, P], BF16, tag="w3a", bufs=1)
            nc.sync.dma_start(w3_first[:], wQ[0, :, 1])
            for lo, hi in X_CHUNKS:
                nc.gpsimd.dma_start(x_sb[:, lo:hi], xQ[:, lo:hi])
            if _rep == 0:
                # PE warmup on a zeroed tile while the first DMAs land:
                # ramps the clock (HAM) so the real chains run full-speed.
                # Junk results land in the ps1 bank the real k=0 matmul
                # clears with start=True.
                warm = tmppool.tile([P, TB], BF16, tag="warm", bufs=1)
                nc.vector.memset(warm[:], 0.0)

            # Stage 1: hT = silu(x @ w1.T).T * (x @ w3.T).T
            for ht in range(KH):
                if ht == 0:
                    w1s, w3s = w1_first, w3_first
                else:
                    w_sb = wpool.tile([P, 2, KD, P], BF16, tag="w")
                    nc.sync.dma_start(w_sb[:], wQ[ht])
                    w1s, w3s = w_sb[:, 0], w_sb[:, 1]
                    if ht == KH - 1:
                        # w2 rides the SP ring behind the weight stream:
                        # lands well before stage 2, keeps gpsimd free
                        # for the x load and output drain.
                        nc.sync.dma_start(w2_sb[:], w2Q[:])
                for tb in range(N_TB):
                    ps1 = pspool.tile([P, TB], F32, tag="ps1", bufs=3)
                    ps3 = pspool.tile([P, TB], F32, tag="ps3", bufs=3)
                    if _rep == 0 and ht == 0 and tb == 0:
                        for _ in range(3):
                            nc.tensor.matmul(
                                ps1[:], warm[:, 0:P], warm[:],
                                start=True, stop=True,
                            )
                    for k in range(KD):
                        nc.tensor.matmul(
                            ps1[:], w1s[:, k], x_sb[:, k, ts(tb, TB)],
                            start=(k == 0), stop=(k == KD - 1),
                        )
                        nc.tensor.matmul(
                            ps3[:], w3s[:, k], x_sb[:, k, ts(tb, TB)],
                            start=(k == 0), stop=(k == KD - 1),
                        )
                    sil = tmppool.tile([P, TB], F32, tag="sil")
                    nc.scalar.activation(
                        sil[:], ps1[:], mybir.ActivationFunctionType.Silu
                    )
                    nc.vector.tensor_mul(h_sb[:, ht, ts(tb, TB)], sil[:], ps3[:])

            # Stage 2: out = hT.T @ w2.T — stationary hT token-tiles,
            # moving w2 dim-blocks, all of w2 resident in SBUF. Output
            # rows accumulate in SBUF and leave as one 512 KB DMA per
            # token-tile on the gpsimd ring.
            for tt in range(N_TT):
                ot = opool.tile([P, DIM], BF16, tag="ot", bufs=2)
                for db in range(N_DB):
                    ps = pspool.tile([P, DB], F32, tag="ps2")
                    for k in range(KH):
                        nc.tensor.matmul(
                            ps[:], h_sb[:, k, ts(tt, P)], w2_sb[:, k, ts(db, DB)],
                            start=(k == 0), stop=(k == KH - 1),
                        )
                    if db % 2 == 0:
                        nc.vector.tensor_copy(ot[:, ts(db, DB)], ps[:])
                    else:
                        nc.scalar.activation(
                            ot[:, ts(db, DB)], ps[:],
                            mybir.ActivationFunctionType.Copy,
                        )
                nc.gpsimd.dma_start(out[ts(tt, P)], ot[:])

    nc.compile()
    return nc


def _get_nc():
    global _CACHED_NC
    if _CACHED_NC is None:
        _CACHED_NC = _build_nc()
    return _CACHED_NC


def _pack_inputs(x, w1, w2, w3, read_starts):
    """Per-core input dicts, bf16, DMA-optimal (partition-major) layouts."""
    in_maps = []
    for e in range(N_EXPERTS):
        s = int(read_starts[e])
        xe = x[s : s + CAP].astype(NP_BF16)  # [CAP, DIM]
        xQ = np.ascontiguousarray(xe.T.reshape(KD, P, CAP).transpose(1, 0, 2))
        w1Q = w1[e].astype(NP_BF16).T.reshape(KD, P, KH, P).transpose(2, 1, 0, 3)
        w3Q = w3[e].astype(NP_BF16).T.reshape(KD, P, KH, P).transpose(2, 1, 0, 3)
        wQ = np.ascontiguousarray(np.stack([w1Q, w3Q], axis=2))  # [KH,P,2,KD,P]
        w2Q = np.ascontiguousarray(
            w2[e].astype(NP_BF16).T.reshape(KH, P, DIM).transpose(1, 0, 2)
        )
        in_maps.append({"xQ": xQ, "wQ": wQ, "w2Q": w2Q})
    return in_maps


def kernel(x, num_tokens_per_expert, w1, w2, w3):
    x = np.ascontiguousarray(np.asarray(x, dtype=np.float32))
    w1 = np.asarray(w1, dtype=np.float32)
    w2 = np.asarray(w2, dtype=np.float32)
    w3 = np.asarray(w3, dtype=np.float32)
    counts = np.asarray(num_tokens_per_expert).astype(np.int64)

    offsets = np.cumsum(counts)
    starts = offsets - counts
    # jax.lax.dynamic_slice clamps the read start so the slice is in-bounds.
    read_starts = np.clip(starts, 0, N_TOKENS - CAP)

    in_maps = _pack_inputs(x, w1, w2, w3, read_starts)
    nc = _get_nc()
    res = run_bass_kernel_spmd(nc, in_maps, core_ids=list(range(N_EXPERTS)))
    ye = [np.asarray(res.results[e]["out"]).astype(np.float32) for e in range(N_EXPERTS)]

    if np.all(counts == CAP):
        # balanced routing: per-expert tiles are disjoint and exactly cover x
        return np.concatenate(ye, axis=0)

    # general case: mask invalid slots, scatter-add to clipped positions
    y = np.zeros((N_TOKENS, DIM), np.float32)
    slot = np.arange(CAP)
    for e in range(N_EXPERTS):
        valid = slot < counts[e]
        pos = np.clip(starts[e] + slot, 0, N_TOKENS - 1)
        np.add.at(y, pos, np.where(valid[:, None], ye[e], 0.0))
    return y
